# revision 1
# baseline (speedup 1.0000x reference)
"""Gaussian mixture loss on 8 Trainium2 NeuronCores (Bass/Tile).

Math: for each predicted point p and gt means g_m,
    ll(p) = logsumexp_m( -C - ||p - g_m||^2 / 2 ),   C = 0.5*log(2*pi)
    loss  = -mean(ll)
Since all exponents are <= -C, exp never overflows and underflow is
harmless, so no max-subtraction is needed:
    ll(p) = -C + log( sum_m exp(p.g_m - ||g_m||^2/2 - ||p||^2/2) )

Kernel strategy (per core):
  - core c handles batch b=c//2, rows (c%2)*2048..+2048, all 4096 gt means
  - t[n,m] = 2 p.g - ||g||^2 - ||p||^2 via ONE K=21 fp16 matmul per
    [128 x 512] tile (pairing index k):
      k 0-2:   lhsT (2p)_hi     | rhs g_hi
      k 3-5:   lhsT (2p)_lo     | rhs g_hi
      k 6-8:   lhsT (2p)_hi     | rhs g_lo
      k 9-14:  lhsT 1           | rhs (-g_d^2)_hi, (-g_d^2)_lo
      k 15-20: lhsT (-p_d^2)_hi, (-p_d^2)_lo | rhs 1
    (hi/lo fp16 splits keep ~2^-22 relative accuracy; matmul cost is
     K-independent, so K=21 is free)
  - exp(0.5*t) + row-sum fused in one ACT instruction per 2048 cols
    (scale=0.5 applied in fp32 inside ACT; accum_out does the row sum)
  - ln + row-sum fused the same way; partition_all_reduce; scalar out
Host: loss = C - (sum of 8 partial sums) / 16384.
"""

import numpy as np

import concourse.bacc as bacc_mod
import concourse.tile as tile
from concourse import bacc, hw_specs, mybir
from concourse.bass_isa import ReduceOp
from concourse.bass_utils import run_bass_kernel_spmd
from concourse.masks import make_identity


def _patched_activation_tables(module_arch):
    """Steer Bacc's act-table-load chooser to the one set that contains
    BOTH Exp and Ln ("natural_log_exp_and_others"), so the kernel pays a
    single ACT_TABLE_LOAD at t=0 instead of an exp load at start plus a
    ~1.3us ln load on the critical tail. Dict order (and therefore the
    act_func_set_id <-> act_info.json index mapping) is preserved; only
    Exp/Ln membership in the other sets is masked for the chooser."""
    both = {mybir.ActivationFunctionType.Exp, mybir.ActivationFunctionType.Ln}
    out = {}
    for name, funcs in hw_specs.get_activation_tables(module_arch).items():
        if name != "natural_log_exp_and_others":
            funcs = set(funcs) - both
        out[name] = funcs
    return out

# Problem shape (hardcoded per contract)
B, N, M, D = 4, 4096, 4096, 3
NCORES = 8
CORES_PER_BATCH = NCORES // B          # 2
RPC = N // CORES_PER_BATCH             # 2048 rows per core
CONST = 0.5 * np.log(2.0 * np.pi)

P = 128                                # partitions
NP_CH = RPC // P                       # 16 p-chunks per core
NG_CH = M // P                         # 32 g-chunks per core
KAUG = 21                              # augmented contraction dim
MMN = 512                              # matmul moving free dim (1 PSUM bank fp32)
NRHS = M // MMN                        # 8 rhs tiles of [21, 512]
HALF = M // 2                          # 2048 cols per ACT instruction
# PSUM layout: 2 "mm" slots of 4 banks (2048 fp32) = all 8 banks. The
# fp16 transposes go through the DMA xbar (SBUF->SBUF), not PSUM.

F32 = mybir.dt.float32
F16 = mybir.dt.float16


TB = 4                                 # chunks preprocessed per batch


def _build_g_batch(nc, g_all, work, zeros, b, raw=None):
    """DVE-only build of g-chunks 4b..4b+3: 7 batched DVE ops reading a
    slice of the bulk-loaded g_all. Returns the aug tile; per-chunk
    transposes are emitted separately."""
    if raw is None:
        raw = g_all[:, TB * b:TB * (b + 1), :]

    neg = work.tile([P, TB, D], F32, tag="neg")
    nc.vector.tensor_sub(out=neg, in0=zeros, in1=raw)              # -g
    nsq = work.tile([P, TB, D], F32, tag="nsq")
    nc.vector.tensor_mul(out=nsq, in0=raw, in1=neg)                # -g_d^2

    aug = work.tile([P, TB, 32], F16, tag="aug", name=f"aug_g{b}", bufs=6)
    nc.vector.tensor_copy(out=aug[:, :, 0:3], in_=raw)             # g_hi
    nc.vector.tensor_copy(out=aug[:, :, 3:6], in_=aug[:, :, 0:3])  # g_hi dup
    nc.vector.tensor_sub(out=aug[:, :, 6:9], in0=raw, in1=aug[:, :, 0:3])
    nc.vector.tensor_copy(out=aug[:, :, 9:12], in_=nsq)            # (-g^2)_hi
    nc.vector.tensor_sub(out=aug[:, :, 12:15], in0=nsq, in1=aug[:, :, 9:12])
    nc.vector.memset(aug[:, :, 15:21], 1.0)                        # ones
    return aug


def _build_p_batch(nc, p_all, work, zeros, b, raw=None, eng=None):
    """Build of p-chunks 4b..4b+3: hi/lo of 2p + neg squares. Runs on
    DVE by default; the opening batch runs on the idle GPSIMD so it
    doesn't serialize behind g0's build on the first-slice chain."""
    if raw is None:
        raw = p_all[:, TB * b:TB * (b + 1), :]
    if eng is None:
        eng = nc.vector

    p2 = work.tile([P, TB, D], F32, tag="p2")
    eng.tensor_add(out=p2, in0=raw, in1=raw)                       # 2p
    neg = work.tile([P, TB, D], F32, tag="neg")
    eng.tensor_sub(out=neg, in0=zeros, in1=raw)                    # -p
    nsq = work.tile([P, TB, D], F32, tag="nsq")
    eng.tensor_mul(out=nsq, in0=raw, in1=neg)                      # -p_d^2

    aug = work.tile([P, TB, 32], F16, tag="aug", name=f"aug_p{b}", bufs=6)
    eng.tensor_copy(out=aug[:, :, 0:3], in_=p2)                    # (2p)_hi
    eng.tensor_sub(out=aug[:, :, 3:6], in0=p2, in1=aug[:, :, 0:3])
    eng.tensor_copy(out=aug[:, :, 6:9], in_=aug[:, :, 0:3])        # hi dup
    eng.memset(aug[:, :, 9:15], 1.0)                               # ones
    eng.tensor_copy(out=aug[:, :, 15:18], in_=nsq)                 # (-p^2)_hi
    eng.tensor_sub(out=aug[:, :, 18:21], in0=nsq, in1=aug[:, :, 15:18])
    return aug


def build_program():
    nc = bacc.Bacc(
        "TRN2",
        target_bir_lowering=False,
        debug=False,
        num_devices=NCORES,
    )
    pred_h = nc.dram_tensor("pred", [RPC, D], F32, kind="ExternalInput").ap()
    gt_h = nc.dram_tensor("gt", [M, D], F32, kind="ExternalInput").ap()
    out_h = nc.dram_tensor("out", [1, 1], F32, kind="ExternalOutput").ap()

    pred_view = pred_h.rearrange("(c p) d -> p c d", p=P)   # [128, 16, 3]
    gt_view = gt_h.rearrange("(c p) d -> p c d", p=P)       # [128, 32, 3]

    with tile.TileContext(nc) as tc:
        with (
            tc.tile_pool(name="consts", bufs=1) as consts,
            tc.tile_pool(name="work", bufs=3) as work,
            tc.tile_pool(name="psum", bufs=2, space="PSUM") as psum,
        ):
            # Fast-path loads for the first g/p batch, plus bulk loads
            # of everything (one DMA each) that later batches slice.
            raw_g0 = work.tile([P, TB, D], F32, tag="rawg0")
            nc.sync.dma_start(out=raw_g0, in_=gt_view[:, 0:TB, :])
            raw_p0 = work.tile([P, TB, D], F32, tag="rawp0")
            nc.sync.dma_start(out=raw_p0, in_=pred_view[:, 0:TB, :])
            g_all = consts.tile([P, NG_CH, D], F32, tag="gall")
            nc.sync.dma_start(out=g_all, in_=gt_view)
            p_all = consts.tile([P, NP_CH, D], F32, tag="pall")
            nc.sync.dma_start(out=p_all, in_=pred_view)

            zeros = consts.tile([P, TB, D], F32, tag="zeros")
            nc.vector.memset(zeros, 0.0)
            identity = consts.tile([P, P], F16, tag="ident")
            make_identity(nc, identity)

            # Warm the ACT exp table at t=0 so the ~1.3us
            # PSEUDO_LOAD_ACT_FUNC_SET overlaps preprocessing instead of
            # stalling the first real exp.
            warm = consts.tile([P, 1], F32, tag="warm")
            nc.vector.memset(warm, 1.0)
            nc.scalar.activation(out=warm, in_=warm,
                                 func=mybir.ActivationFunctionType.Exp)

            rhs_tiles = [consts.tile([KAUG, MMN], F16, tag=f"rh{j}", name=f"rh{j}")
                         for j in range(NRHS)]
            lhsT_tiles = [consts.tile([KAUG, P], F16, tag=f"lh{i}", name=f"lh{i}")
                          for i in range(NP_CH)]
            # acc: 2 blocks of 16 (half h at cols h*16+i) + 3 scratch
            # cols for the finely-split first group
            NACC = 2 * NP_CH
            acc = consts.tile([P, NACC + 3], F32, tag="acc")

            # Per-chunk transpose tasks, drained a couple at a time
            # between main groups so the psum-slot traffic hides in the
            # ~2us ACT-drain shadow instead of bursting.
            pending = []

            def tr_task(aug, t, dst):
                def run():
                    ps = psum.tile([KAUG, P], F16, tag="mm", name="trps")
                    nc.tensor.transpose(ps, aug[:, t, 0:KAUG], identity)
                    nc.vector.tensor_copy(out=dst, in_=ps)
                pending.append(run)

            def drain(n):
                for _ in range(min(n, len(pending))):
                    pending.pop(0)()

            def queue_g(aug, b):
                rt = rhs_tiles[b]
                for t in range(TB):
                    tr_task(aug, t, rt[:, t * P:(t + 1) * P])

            def queue_p(aug, b):
                for t in range(TB):
                    tr_task(aug, t, lhsT_tiles[TB * b + t])

            def main_group(i, h):
                pt = psum.tile([P, HALF], F32, tag="mm")
                for q in range(NRHS // 2):
                    nc.tensor.matmul(
                        pt[:, q * MMN:(q + 1) * MMN],
                        lhsT=lhsT_tiles[i],
                        rhs=rhs_tiles[h * (NRHS // 2) + q],
                        start=True, stop=True,
                    )
                # exp to SBUF fp16 (skips ACT's 187ns accumulator read);
                # the row-sum runs on DVE in 4x fp16 mode instead.
                ex = work.tile([P, HALF], F16, tag="ex", bufs=3)
                nc.scalar.activation(
                    out=ex, in_=pt, func=mybir.ActivationFunctionType.Exp,
                    bias=0.0, scale=0.5,
                )
                col = h * NP_CH + i
                nc.vector.tensor_scalar(
                    out=ex, in0=ex, scalar1=1.0, scalar2=0.0,
                    op0=mybir.AluOpType.mult, op1=mybir.AluOpType.add,
                    accum_out=acc[:, col:col + 1],
                )

            # Opening sequence: chunk 0 runs 512-wide ACT slices on its
            # own single-bank psum tiles, each emitted right after its
            # rhs tile is transposed, so exp starts as soon as rhs0
            # alone is ready.
            aug_g0 = _build_g_batch(nc, g_all, work, zeros, 0, raw=raw_g0)
            queue_g(aug_g0, 0)
            aug_p0 = _build_p_batch(nc, p_all, work, zeros, 0, raw=raw_p0)
            tr_task(aug_p0, 0, lhsT_tiles[0])
            drain(5)                      # rhs0 cols + lhsT0
            for q in range(4):
                ptq = psum.tile([P, MMN], F32, tag="mm", name=f"pt0_{q}")
                nc.tensor.matmul(
                    ptq, lhsT=lhsT_tiles[0], rhs=rhs_tiles[q],
                    start=True, stop=True,
                )
                col = [NACC, NACC + 1, NACC + 2, 0][q]
                nc.scalar.activation(
                    out=ptq, in_=ptq,
                    func=mybir.ActivationFunctionType.Exp,
                    bias=0.0, scale=0.5,
                    accum_out=acc[:, col:col + 1],
                )
                if q < 3:
                    queue_g(_build_g_batch(nc, g_all, work, zeros, q + 1),
                            q + 1)
                    drain(4)              # rhs q+1
                    # lhsT q+1 early, so group 1..3 matmuls can start
                    # the moment the opening slices finish
                    tr_task(aug_p0, q + 1, lhsT_tiles[q + 1])
                    drain(1)
            queue_p(_build_p_batch(nc, p_all, work, zeros, 1), 1)

            for i in range(1, NP_CH):
                drain(2)
                main_group(i, 0)
                if i == 2:
                    queue_g(_build_g_batch(nc, g_all, work, zeros, 4), 4)
                elif i == 4:
                    queue_p(_build_p_batch(nc, p_all, work, zeros, 2), 2)
                elif i == 6:
                    queue_g(_build_g_batch(nc, g_all, work, zeros, 5), 5)
                elif i == 8:
                    queue_p(_build_p_batch(nc, p_all, work, zeros, 3), 3)
                elif i == 10:
                    queue_g(_build_g_batch(nc, g_all, work, zeros, 6), 6)
                elif i == 12:
                    queue_g(_build_g_batch(nc, g_all, work, zeros, 7), 7)
            for i in range(NP_CH):
                drain(3)
                main_group(i, 1)
            drain(len(pending))

            # ---- finalize: S = acc_h0 + acc_h1 (+ first-group parts) ----
            S = consts.tile([P, NP_CH], F32, tag="S")
            nc.vector.tensor_add(S, acc[:, 0:NP_CH],
                                 acc[:, NP_CH:2 * NP_CH])
            for e in range(3):
                nc.vector.tensor_add(S[:, 0:1], S[:, 0:1],
                                     acc[:, NACC + e:NACC + e + 1])
            LL = consts.tile([P, NP_CH], F32, tag="LL")
            rowsum = consts.tile([P, 1], F32, tag="rowsum")
            nc.scalar.activation(
                out=LL, in_=S, func=mybir.ActivationFunctionType.Ln,
                accum_out=rowsum,
            )
            red = consts.tile([P, 1], F32, tag="red")
            nc.gpsimd.partition_all_reduce(red, rowsum, P, ReduceOp.add)
            nc.sync.dma_start(out=out_h[0:1, 0:1], in_=red[0:1, 0:1])

    orig_tables = bacc_mod.get_activation_tables
    bacc_mod.get_activation_tables = _patched_activation_tables
    try:
        nc.compile()
    finally:
        bacc_mod.get_activation_tables = orig_tables
    return nc


_NC_CACHE = {}


def run(predicted_points, gt_means, trace=False, **trace_kwargs):
    """Shard inputs, run the SPMD bass kernel, gather. Returns
    (loss_scalar_f32, BassKernelResults)."""
    pred = np.ascontiguousarray(np.asarray(predicted_points, dtype=np.float32))
    gt = np.ascontiguousarray(np.asarray(gt_means, dtype=np.float32))
    assert pred.shape == (B, N, D) and gt.shape == (B, M, D)

    if "nc" not in _NC_CACHE:
        _NC_CACHE["nc"] = build_program()
    nc = _NC_CACHE["nc"]

    in_maps = []
    for c in range(NCORES):
        b = c // CORES_PER_BATCH
        r0 = (c % CORES_PER_BATCH) * RPC
        in_maps.append({
            "pred": np.ascontiguousarray(pred[b, r0:r0 + RPC, :]),
            "gt": np.ascontiguousarray(gt[b]),
        })

    res = run_bass_kernel_spmd(nc, in_maps, list(range(NCORES)),
                               trace=trace, **trace_kwargs)
    total = 0.0
    for c in range(NCORES):
        total += float(res.results[c]["out"][0, 0])
    loss = np.asarray(CONST - total / (B * N), dtype=np.float32)
    return loss, res


def kernel(predicted_points, gt_means):
    loss, _ = run(predicted_points, gt_means, trace=False)
    return loss



# revision 8
# speedup vs baseline: 1.0922x; 1.0922x over previous
"""Gaussian mixture loss on 8 Trainium2 NeuronCores (Bass/Tile).

Math: for each predicted point p and gt means g_m,
    ll(p) = logsumexp_m( -C - ||p - g_m||^2 / 2 ),   C = 0.5*log(2*pi)
    loss  = -mean(ll)
All exponents are <= 0, so exp never overflows and underflow is harmless:
    ll(p) = ln( sum_m 2^(y_m) ),  y = ALPHA*(2 p.g - |g|^2 - |p|^2),
    ALPHA = 0.5*log2(e)  (folded into the matmul operands).

Kernel strategy (per core):
  - core c handles batch b=c//2, rows (c%2)*2048..+2048, all 4096 gt means
  - y[n,m] via ONE K=32 fp16 matmul per [128 x 512] tile (pairing index k):
      k 0-2: lhsT 2*ALPHA*p_d   | rhs g_d
      k 3:   lhsT ALPHA         | rhs -|g|^2
      k 4:   lhsT -ALPHA*|p|^2  | rhs 1
      k 5-31: zeros (pad: 3 chunks of 32 per [128,128] DMA transpose, at
              the PE-legal base partitions 0/32/64)
  - exp2 + row-sum of each [128 x 2048] PSUM half is SPLIT across two
    engines to beat the single-engine activation roofline:
      * ACT halves: one activation (Exp, scale=ln2) in-place on PSUM with
        accum_out doing the row sum.
      * DVE halves: truncating Schraudolph exp2 — one tensor_scalar
        (y + (127+SIGMA)) * 2^23 written through the fp32->int32 output
        converter gives the BIT PATTERN of 2^y with linear-mantissa
        interpolation; a second tensor_scalar over the fp32-bitcast view
        row-sums via accum_out. SIGMA centers the log-domain interp bias
        so the row-sum is unbiased (tuned: rel err ~1e-5 end to end).
  - transposes: aug tiles are built row-major (cheap batched DVE ops)
    and transposed by DmaTransposeAnt [128x128]; lhsT chunks are direct
    row-slices of the transposed tiles; the g-side is repacked into
    [32 x 512] rhs tiles by GPSIMD SBUF->SBUF copies.
  - ln + row-sum fused in one ACT instruction; partition_all_reduce.
Host: loss = C - (sum of 8 partial sums) / 16384.
"""

import numpy as np

import concourse.bacc as bacc_mod
import concourse.tile as tile
from concourse import bacc, hw_specs, mybir
from concourse.bass_isa import ReduceOp
from concourse.bass_utils import run_bass_kernel_spmd


def _patched_activation_tables(module_arch):
    """Steer Bacc's act-table-load chooser to the one set that contains
    BOTH Exp and Ln ("natural_log_exp_and_others"), so the kernel pays a
    single ACT_TABLE_LOAD at t=0 instead of an exp load at start plus a
    ~1.3us ln load on the critical tail."""
    both = {mybir.ActivationFunctionType.Exp, mybir.ActivationFunctionType.Ln}
    out = {}
    for name, funcs in hw_specs.get_activation_tables(module_arch).items():
        if name != "natural_log_exp_and_others":
            funcs = set(funcs) - both
        out[name] = funcs
    return out

# Problem shape (hardcoded per contract)
B, N, M, D = 4, 4096, 4096, 3
NCORES = 8
CORES_PER_BATCH = NCORES // B          # 2
RPC = N // CORES_PER_BATCH             # 2048 rows per core
CONST = 0.5 * np.log(2.0 * np.pi)

P = 128                                # partitions
NP_CH = RPC // P                       # 16 p-chunks per core
NG_CH = M // P                         # 32 g-chunks per core
K32 = 32                               # padded contraction dim
MMN = 512                              # matmul moving free dim (1 PSUM bank)
HALF = M // 2                          # 2048 cols per consumer instruction
TP = (NP_CH + 2) // 3                  # 6 p-side transposes (3 chunks each)
TG = (NG_CH + 2) // 3                  # 11 g-side transposes

ALPHA = 0.5 * np.log2(np.e)            # exp(t/2) = 2^(ALPHA*t)
SIGMA = -0.060                         # Schraudolph bias centering (tuned)
MAGIC = float(np.float32(127.0 + SIGMA))
SC23 = float(np.float32(2.0 ** 23))
LN2 = float(np.log(2.0))
ND = 13                                # of 32 halves, how many go to DVE

F32 = mybir.dt.float32
F16 = mybir.dt.float16
I32 = mybir.dt.int32


def _dve_half_set():
    """Spread ND DVE-halves evenly over the 32 halves."""
    s = set()
    a = 0
    for idx in range(32):
        a += ND
        if a >= 32:
            a -= 32
            s.add(idx)
    return s


def build_program():
    nc = bacc.Bacc(
        "TRN2",
        target_bir_lowering=False,
        debug=False,
        num_devices=NCORES,
    )
    pred_h = nc.dram_tensor("pred", [RPC, D], F32, kind="ExternalInput").ap()
    gt_h = nc.dram_tensor("gt", [M, D], F32, kind="ExternalInput").ap()
    out_h = nc.dram_tensor("out", [1, 1], F32, kind="ExternalOutput").ap()

    pred_view = pred_h.rearrange("(c p) d -> p c d", p=P)   # [128, 16, 3]
    gt_view = gt_h.rearrange("(c p) d -> p c d", p=P)       # [128, 32, 3]

    dve_set = _dve_half_set()

    with tile.TileContext(nc) as tc:
        with (
            tc.tile_pool(name="consts", bufs=1) as consts,
            tc.tile_pool(name="work", bufs=2) as work,
            tc.tile_pool(name="psum", bufs=2, space="PSUM") as psum,
        ):
            p_all = consts.tile([P, NP_CH, D], F32, tag="pall")
            nc.sync.dma_start(out=p_all, in_=pred_view)
            g_all = consts.tile([P, NG_CH, D], F32, tag="gall")
            nc.sync.dma_start(out=g_all, in_=gt_view)

            # Warm the ACT exp table at t=0 so the ~1.3us table load
            # overlaps preprocessing instead of stalling the first exp.
            warm = consts.tile([P, 1], F32, tag="warm")
            nc.vector.memset(warm, 1.0)
            nc.scalar.activation(out=warm, in_=warm,
                                 func=mybir.ActivationFunctionType.Exp)

            # ---- p-side aug [128, TP, 4, 32] f16 (chunk i -> slot
            # (i//3, i%3); slot 3 of each group is zero pad) ----
            paug = consts.tile([P, TP, 4, K32], F16, tag="paug")
            nc.vector.memset(paug, 0.0)
            nc.vector.memset(paug[:, :, 0:3, 3:4], float(ALPHA))
            # chunks 0..14 batched, chunk 15 separate
            nc.vector.tensor_scalar(
                out=paug[:, 0:5, 0:3, 0:3],
                in0=p_all[:, 0:15, :].rearrange("p (t r) d -> p t r d", r=3),
                scalar1=float(2.0 * ALPHA), scalar2=0.0,
                op0=mybir.AluOpType.mult, op1=mybir.AluOpType.add)
            nc.vector.tensor_scalar(
                out=paug[:, 5, 0:1, 0:3], in0=p_all[:, 15:16, :],
                scalar1=float(2.0 * ALPHA), scalar2=0.0,
                op0=mybir.AluOpType.mult, op1=mybir.AluOpType.add)
            sqp = work.tile([P, NP_CH, D], F32, tag="sqp")
            nc.vector.tensor_mul(out=sqp, in0=p_all, in1=p_all)
            rp = work.tile([P, NP_CH, 1], F32, tag="rp")
            nc.vector.tensor_reduce(out=rp, in_=sqp, op=mybir.AluOpType.add,
                                    axis=mybir.AxisListType.X)
            nc.vector.tensor_scalar(
                out=paug[:, 0:5, 0:3, 4:5],
                in0=rp[:, 0:15, :].rearrange("p (t r) d -> p t r d", r=3),
                scalar1=float(-ALPHA), scalar2=0.0,
                op0=mybir.AluOpType.mult, op1=mybir.AluOpType.add)
            nc.vector.tensor_scalar(
                out=paug[:, 5, 0:1, 4:5], in0=rp[:, 15:16, :],
                scalar1=float(-ALPHA), scalar2=0.0,
                op0=mybir.AluOpType.mult, op1=mybir.AluOpType.add)

            pT = []
            for t in range(TP):
                pt_t = consts.tile([P, P], F16, tag=f"pT{t}", name=f"pT{t}")
                nc.sync.dma_start_transpose(
                    out=pt_t,
                    in_=paug[:, t].rearrange("p r k -> p (r k)"))
                pT.append(pt_t)
            # matmul needs lhsT and rhs at the SAME base partition — peel
            # each chunk down to a base-0 tile with GPSIMD copies.
            lhsT_tiles = [consts.tile([K32, P], F16, tag=f"lh{i}",
                                      name=f"lh{i}")
                          for i in range(NP_CH)]
            for i in range(NP_CH):
                nc.gpsimd.tensor_copy(
                    out=lhsT_tiles[i],
                    in_=pT[i // 3][(i % 3) * K32:(i % 3 + 1) * K32, :])

            # ---- g-side aug [128, TG, 4, 32] f16 ----
            gaug = consts.tile([P, TG, 4, K32], F16, tag="gaug")
            nc.vector.memset(gaug, 0.0)
            nc.vector.memset(gaug[:, :, 0:3, 4:5], 1.0)
            nc.vector.tensor_copy(
                out=gaug[:, 0:10, 0:3, 0:3],
                in_=g_all[:, 0:30, :].rearrange("p (t r) d -> p t r d", r=3))
            nc.vector.tensor_copy(
                out=gaug[:, 10, 0:2, 0:3], in_=g_all[:, 30:32, :])
            sqg = work.tile([P, NG_CH, D], F32, tag="sqg")
            nc.vector.tensor_mul(out=sqg, in0=g_all, in1=g_all)
            rg = work.tile([P, NG_CH, 1], F32, tag="rg")
            nc.vector.tensor_reduce(out=rg, in_=sqg, op=mybir.AluOpType.add,
                                    axis=mybir.AxisListType.X)
            nc.vector.tensor_scalar(
                out=gaug[:, 0:10, 0:3, 3:4],
                in0=rg[:, 0:30, :].rearrange("p (t r) d -> p t r d", r=3),
                scalar1=-1.0, scalar2=0.0,
                op0=mybir.AluOpType.mult, op1=mybir.AluOpType.add)
            nc.vector.tensor_scalar(
                out=gaug[:, 10, 0:2, 3:4], in0=rg[:, 30:32, :],
                scalar1=-1.0, scalar2=0.0,
                op0=mybir.AluOpType.mult, op1=mybir.AluOpType.add)

            gT = []
            for t in range(TG):
                gt_t = consts.tile([P, P], F16, tag=f"gT{t}", name=f"gT{t}")
                nc.sync.dma_start_transpose(
                    out=gt_t,
                    in_=gaug[:, t].rearrange("p r k -> p (r k)"))
                gT.append(gt_t)

            # ---- GPSIMD repack: rhs_j [32, 512], j = 0..7 ----
            rhs_tiles = [consts.tile([K32, MMN], F16, tag=f"rh{j}",
                                     name=f"rh{j}")
                         for j in range(8)]
            for j in range(8):
                for c in range(4):
                    gc = 4 * j + c
                    src = gT[gc // 3][(gc % 3) * K32:(gc % 3 + 1) * K32, :]
                    nc.gpsimd.tensor_copy(
                        out=rhs_tiles[j][:, c * P:(c + 1) * P], in_=src)

            acc = consts.tile([P, 2 * NP_CH], F32, tag="acc")
            e16 = consts.tile([P, HALF], F16, tag="e16")    # rowsum scratch

            # ---- main loop: 32 halves over 2 PSUM slots ----
            for idx in range(32):
                i, h = idx % NP_CH, idx // NP_CH
                pt = psum.tile([P, HALF], F32, tag="mm")
                for q in range(4):
                    nc.tensor.matmul(
                        pt[:, q * MMN:(q + 1) * MMN],
                        lhsT=lhsT_tiles[i],
                        rhs=rhs_tiles[4 * h + q],
                        start=True, stop=True,
                    )
                col = h * NP_CH + i
                if idx in dve_set:
                    sch = work.tile([P, HALF], I32, tag="sch", bufs=2)
                    nc.vector.tensor_scalar(
                        out=sch, in0=pt, scalar1=MAGIC, scalar2=SC23,
                        op0=mybir.AluOpType.add, op1=mybir.AluOpType.mult)
                    nc.vector.tensor_scalar(
                        out=e16, in0=sch.bitcast(F32), scalar1=1.0,
                        scalar2=0.0,
                        op0=mybir.AluOpType.mult, op1=mybir.AluOpType.add,
                        accum_out=acc[:, col:col + 1])
                else:
                    nc.scalar.activation(
                        out=pt, in_=pt,
                        func=mybir.ActivationFunctionType.Exp,
                        bias=0.0, scale=LN2,
                        accum_out=acc[:, col:col + 1])

            # ---- finalize: S = acc_h0 + acc_h1; ln; reduce ----
            S = consts.tile([P, NP_CH], F32, tag="S")
            nc.vector.tensor_add(S, acc[:, 0:NP_CH], acc[:, NP_CH:2 * NP_CH])
            LL = consts.tile([P, NP_CH], F32, tag="LL")
            rowsum = consts.tile([P, 1], F32, tag="rowsum")
            nc.scalar.activation(
                out=LL, in_=S, func=mybir.ActivationFunctionType.Ln,
                accum_out=rowsum,
            )
            red = consts.tile([P, 1], F32, tag="red")
            nc.gpsimd.partition_all_reduce(red, rowsum, P, ReduceOp.add)
            nc.sync.dma_start(out=out_h[0:1, 0:1], in_=red[0:1, 0:1])

    orig_tables = bacc_mod.get_activation_tables
    bacc_mod.get_activation_tables = _patched_activation_tables
    try:
        nc.compile()
    finally:
        bacc_mod.get_activation_tables = orig_tables
    return nc


_NC_CACHE = {}


def run(predicted_points, gt_means, trace=False, **trace_kwargs):
    """Shard inputs, run the SPMD bass kernel, gather. Returns
    (loss_scalar_f32, BassKernelResults)."""
    pred = np.ascontiguousarray(np.asarray(predicted_points, dtype=np.float32))
    gt = np.ascontiguousarray(np.asarray(gt_means, dtype=np.float32))
    assert pred.shape == (B, N, D) and gt.shape == (B, M, D)

    if "nc" not in _NC_CACHE:
        _NC_CACHE["nc"] = build_program()
    nc = _NC_CACHE["nc"]

    in_maps = []
    for c in range(NCORES):
        b = c // CORES_PER_BATCH
        r0 = (c % CORES_PER_BATCH) * RPC
        in_maps.append({
            "pred": np.ascontiguousarray(pred[b, r0:r0 + RPC, :]),
            "gt": np.ascontiguousarray(gt[b]),
        })

    res = run_bass_kernel_spmd(nc, in_maps, list(range(NCORES)),
                               trace=trace, **trace_kwargs)
    total = 0.0
    for c in range(NCORES):
        total += float(res.results[c]["out"][0, 0])
    loss = np.asarray(CONST - total / (B * N), dtype=np.float32)
    return loss, res


def kernel(predicted_points, gt_means):
    loss, _ = run(predicted_points, gt_means, trace=False)
    return loss
